# revision 30
# baseline (speedup 1.0000x reference)
"""Conv3d (k=3, pad=1) as shifted-window matmuls on 8 TRN2 NeuronCores.

Sharding: data-parallel over (batch B=2) x (T quarters of 8 output frames).
Each core computes out[b, :, t0:t0+8, :, :] from a host-padded input shard
xs[ci, 10, 130, 130] (conv zero-padding + t-halo baked in by the host).

Per-core formulation: output tile M=128 = (co=32, dt=2, dh=2) output
positions, contraction K=128 = (jt=4 t-window slots, jhg=2 h-parity, ci=16),
free dim = (h-blocks, w). The 3x3x3 kernel becomes 6 accumulating matmuls
(kw=3 x jhh=2) per PSUM bank, with all w/h shifts expressed as free-dim AP
offsets into SBUF-resident tiles.

v3: bf16 activations/weights/outputs (halves HBM traffic, full-rate PE);
outputs stored per 16-block group; PSUM evictions alternate Vector/Scalar
engines; dummy matmuls on a zeroed tile pre-warm the PE HAM clock gate
during the initial DMA wait.

v4: head restructured around measured DMA behavior — the gating transfers
(weights in partition-major 1536B-descriptor chunks, then four 5-row x
slivers) ride the SP HWDGE queue in exact need-order while scalar carries
only the bias; the first group runs bank-major so each PSUM bank starts as
soon as its own sliver lands; warmup count bridges PE-ready to data-ready
with no idle gap (a gap delays the HAM ramp ~1:1). Final group is also
bank-major with per-bank stores on SP (the scalar engine's ~0.5us sem
wake-up is too slow for the critical tail), so the post-last-matmul tail
is one bank's evict+store, not four.
"""

import sys

if "/opt/trn_rl_repo" not in sys.path:
    sys.path.insert(0, "/opt/trn_rl_repo")

import numpy as np
import ml_dtypes

import concourse.bass as bass
import concourse.mybir as mybir
import concourse.tile as tile
from concourse.bass_utils import run_bass_kernel_spmd

BF16 = ml_dtypes.bfloat16

B, C_IN, T, H, W = 2, 16, 32, 128, 128
C_OUT, KS = 32, 3
N_CORES = 8
TSH = T // 4          # output frames per core
NBT = TSH // 2        # bt tiles per core (2 output frames each)
HB = H // 2           # h blocks (dh=2)
NST = 4               # psum groups per bt tile (4 banks of 4 h-blocks each)
N_WARM = 5            # pre-warm matmuls issued during the initial DMA wait


def _split_excess_waits(nc, limit=1):
    """This walrus build accepts at most ONE sync-wait command per
    instruction. Move excess waits onto same-engine single-wait NoOps placed
    immediately before the instruction (identical blocking semantics)."""
    uid = 0
    for f in nc.m.functions:
        for bb in f.blocks:
            out = []
            for inst in bb.instructions:
                si = inst.sync_info
                if si is not None and si.on_wait and len(si.on_wait) > limit:
                    waits = list(si.on_wait)
                    excess, keep = waits[:-limit], waits[-limit:]
                    for k in range(0, len(excess), limit):
                        nop = mybir.InstNoOp(
                            name=f"wait_split_{uid}", ins=[], outs=[],
                            sync_info=mybir.SyncInfo(
                                on_wait=list(excess[k:k + limit]), on_update=[]))
                        nop.engine = inst.engine
                        nc.register_instruction(nop)
                        uid += 1
                        out.append(nop)
                    si.on_wait = keep
                out.append(inst)
            bb.instructions[:] = out
    return nc


def _build_program(split=True):
    nc = bass.Bass()
    f32 = mybir.dt.float32
    bf16 = mybir.dt.bfloat16
    ident = mybir.ActivationFunctionType.Identity
    # Host pre-arranges the shard partition-major so every tile load/store is
    # ONE <=3-dim DMA: xs[f, jhg, ci, bh, w], out[bt, dt, dh, co, bh, w].
    xs = nc.dram_tensor("xs", [TSH + 2, 2, C_IN, HB + 1, W + 2], bf16,
                        kind="ExternalInput")
    # Host packs the weights partition-major [p, i, m] so both chunk DMAs
    # are dense per-partition copies (no gather descriptors).
    wt = nc.dram_tensor("wt", [128, 6, 128], bf16, kind="ExternalInput")
    bi = nc.dram_tensor("bi", [128, 1], f32, kind="ExternalInput")
    out = nc.dram_tensor("out", [NBT, 2, 2, C_OUT, HB, W], bf16,
                         kind="ExternalOutput")

    with tile.TileContext(nc) as tc:
        with tc.tile_pool(name="wpool", bufs=1) as wpool, \
             tc.tile_pool(name="xpool", bufs=2) as xpool, \
             tc.tile_pool(name="opool", bufs=3) as opool, \
             tc.tile_pool(name="pspool", bufs=2, space="PSUM") as pspool:
            # Head: DMA descriptor size rules effective bandwidth (~25 GB/s
            # per engine at >=4KB descriptors, overhead-dominated below) and
            # the SP HWDGE queue dispatches ~1us sooner than scalar's. Put
            # the two gating transfers (whole w: 1536B descriptors, then the
            # first 5-row x sliver) on SP; the later bank slivers and bias
            # ride the scalar queue and interleave at engine level.
            src0 = xs[0:4].rearrange("f j c b w -> (f j c) b w")
            w_sb = wpool.tile([128, 6, 128], bf16)
            xc = [xpool.tile([128, 5, W + 2], bf16, name=f"xc{j}", bufs=1)
                  for j in range(4)]
            # Everything the first bank needs rides the SP queue in exact
            # need-order (packets drain in issue order; a queue's first
            # bytes are never stuck behind later transfers). Scalar carries
            # only the bias so the head transfers get the engines to
            # themselves.
            nc.sync.dma_start(out=w_sb[:, 0:2, :], in_=wt[:, 0:2, :])
            nc.sync.dma_start(out=xc[0][:, :, :], in_=src0[:, 0:5, :])
            nc.sync.dma_start(out=w_sb[:, 2:6, :], in_=wt[:, 2:6, :])
            for j in range(1, 4):
                nc.sync.dma_start(out=xc[j][:, :, :],
                                  in_=src0[:, 4 * j:4 * j + 5, :])
            b_sb = wpool.tile([128, 1], f32)
            nc.scalar.dma_start(out=b_sb[:, :], in_=bi[:, :])

            # PE pre-warm: the HAM clock gate holds the PE at ~1.2 GHz until
            # ~4.5us of SUSTAINED activity, and an idle gap resets the ramp.
            # Bridge from PE-ready (~7.4us) to first-data (~10.3us) with
            # dummy matmuls on a zeroed tile (the memset runs pre-barrier on
            # gpsimd, off the critical path).
            z_t = wpool.tile([128, 4, 128], bf16)
            nc.gpsimd.memset(z_t[:, :, :], 0)
            ps_w = pspool.tile([128, 4, W], f32, name="ps0")
            for _ in range(N_WARM):
                nc.tensor.matmul(ps_w[:, :, :], z_t[:, 0, :], z_t[:, :, :],
                                 start=True, stop=True)

            for bt in range(NBT):
                # Quarter-height x tiles: group g reads rows [16g, 16g+17)
                # (one shared halo row between quarters).
                src = xs[2 * bt:2 * bt + 4].rearrange(
                    "f j c b w -> (f j c) b w")
                x_q = []
                for q in range(4):
                    if bt == 0 and q == 0:
                        x_q.append(None)    # bt0 g0 reads the xc slivers
                        continue
                    xt = xpool.tile([128, 17, W + 2], bf16, name=f"x{q}")
                    nc.sync.dma_start(out=xt[:, :, :],
                                      in_=src[:, 16 * q:16 * q + 17, :])
                    x_q.append(xt)

                dst = out[bt].rearrange("dt dh co b w -> (dt dh co) b w")
                for g in range(NST):
                    xt = x_q[g]
                    last = bt == NBT - 1 and g == NST - 1
                    pss = [pspool.tile([128, 4, W], f32, name=f"ps{j}")
                           for j in range(4)]
                    if xt is None:
                        # bt0 g0: bank-major order so bank j runs as soon as
                        # its own 5-row sliver lands.
                        for j in range(4):
                            for i in range(6):
                                kw, jhh = divmod(i, 2)
                                rhs = xc[j][:, jhh:jhh + 4, kw:kw + W]
                                nc.tensor.matmul(pss[j][:, :, :],
                                                 w_sb[:, i, :], rhs,
                                                 start=(i == 0), stop=(i == 5))
                    elif last:
                        # Final group: bank-major so early banks evict and
                        # store while later banks still accumulate — the
                        # post-last-matmul tail is one bank, not four.
                        for j in range(4):
                            for i in range(6):
                                kw, jhh = divmod(i, 2)
                                rr = 4 * j + jhh
                                rhs = xt[:, rr:rr + 4, kw:kw + W]
                                nc.tensor.matmul(pss[j][:, :, :],
                                                 w_sb[:, i, :], rhs,
                                                 start=(i == 0), stop=(i == 5))
                    else:
                        for i in range(6):
                            kw, jhh = divmod(i, 2)
                            lhsT = w_sb[:, i, :]
                            for j in range(4):
                                rr = 4 * j + jhh
                                rhs = xt[:, rr:rr + 4, kw:kw + W]
                                nc.tensor.matmul(pss[j][:, :, :], lhsT, rhs,
                                                 start=(i == 0), stop=(i == 5))
                    og = opool.tile([128, 16, W], bf16, name="og")
                    if last:
                        # Tail: banks store as they retire, all on SP (the
                        # scalar engine's sem wake-up is ~0.5us — too slow
                        # for the critical tail). Vector (40ns wake) evicts
                        # banks 0, 2 and the final bank 3.
                        nc.vector.tensor_scalar_add(
                            og[:, 0:4, :], pss[0][:, :, :], b_sb[:, 0:1])
                        nc.scalar.activation(
                            og[:, 4:8, :], pss[1][:, :, :], ident,
                            bias=b_sb[:, 0:1])
                        nc.sync.dma_start(out=dst[:, 16 * g:16 * g + 8, :],
                                          in_=og[:, 0:8, :])
                        nc.vector.tensor_scalar_add(
                            og[:, 8:12, :], pss[2][:, :, :], b_sb[:, 0:1])
                        nc.sync.dma_start(
                            out=dst[:, 16 * g + 8:16 * g + 12, :],
                            in_=og[:, 8:12, :])
                        nc.vector.tensor_scalar_add(
                            og[:, 12:16, :], pss[3][:, :, :], b_sb[:, 0:1])
                        nc.sync.dma_start(
                            out=dst[:, 16 * g + 12:16 * g + 16, :],
                            in_=og[:, 12:16, :])
                    else:
                        for j in range(4):
                            ot = og[:, 4 * j:4 * j + 4, :]
                            if j % 2 == 0:
                                nc.vector.tensor_scalar_add(
                                    ot, pss[j][:, :, :], b_sb[:, 0:1])
                            else:
                                nc.scalar.activation(
                                    ot, pss[j][:, :, :], ident,
                                    bias=b_sb[:, 0:1])
                        nc.sync.dma_start(out=dst[:, 16 * g:16 * g + 16, :],
                                          in_=og[:, :, :])
    _strip_teardown(nc)
    if split:
        _split_excess_waits(nc)
    return nc


def _strip_teardown(nc):
    """Drop the TileContext-exit semaphore RANGE_CLEAR and the second
    all-engine barrier from the end block. They only matter if the NEFF were
    re-executed with dirty semaphore state — each kernel() call compiles and
    runs a fresh single-shot program — and their serial per-semaphore resets
    add ~8us inside the profiled execution window. (Stripping the FIRST
    barrier too was tried and measured worse: the NEFF per-engine epilogue
    storms then overlap and slow the final store/drain chain.)"""
    for f in nc.m.functions:
        for bb in f.blocks:
            if not bb.name.endswith("_end"):
                continue
            insts = bb.instructions
            # Keep everything through the first all-engine barrier (its last
            # instruction is the second consecutive Pool EventSemaphore);
            # drop the Pool drain + RANGE_CLEAR ISA + second barrier.
            for k, inst in enumerate(insts):
                if type(inst).__name__ == "InstISA":
                    start = k
                    while start > 0 and type(insts[start - 1]).__name__ == \
                            "InstDrain":
                        start -= 1
                    bb.instructions[:] = insts[:start]
                    break
    return nc


_NC_CACHE = []


def _get_nc():
    if not _NC_CACHE:
        _NC_CACHE.append(_build_program())
    return _NC_CACHE[0]


def _pack_weights(weight):
    wt = np.zeros((6, 128, 128), np.float32)
    for kw in range(3):
        for jhh in range(2):
            i = kw * 2 + jhh
            for jt in range(4):
                for jhg in range(2):
                    jh = 2 * jhh + jhg
                    r0 = jt * 32 + jhg * 16
                    for dt in range(2):
                        kt = jt - dt
                        if not 0 <= kt < KS:
                            continue
                        for dh in range(2):
                            kh = jh - dh
                            if not 0 <= kh < KS:
                                continue
                            c0 = dt * 64 + dh * 32
                            wt[i, r0:r0 + 16, c0:c0 + 32] = \
                                weight[:, :, kt, kh, kw].T
    # partition-major [p, i, m] so the device-side chunk DMAs are dense
    return np.ascontiguousarray(wt.transpose(1, 0, 2)).astype(BF16)


def run(x, weight, bias, trace=False):
    x = np.asarray(x, dtype=np.float32)
    weight = np.asarray(weight, dtype=np.float32)
    bias = np.asarray(bias, dtype=np.float32)

    xp = np.zeros((B, C_IN, T + 2, H + 2, W + 2), BF16)
    xp[:, :, 1:-1, 1:-1, 1:-1] = x.astype(BF16)
    wt = _pack_weights(weight)
    bi = np.tile(bias, 4).reshape(128, 1).astype(np.float32)

    in_maps = []
    for c in range(N_CORES):
        b, q = divmod(c, 4)
        t0 = q * TSH
        sh = xp[b, :, t0:t0 + TSH + 2]                # [ci, f, 130, 130]
        sh = sh.reshape(C_IN, TSH + 2, HB + 1, 2, W + 2)
        sh = np.ascontiguousarray(sh.transpose(1, 3, 0, 2, 4))
        in_maps.append({"xs": sh, "wt": wt, "bi": bi})

    nc = _get_nc()
    res = run_bass_kernel_spmd(nc, in_maps, list(range(N_CORES)), trace=trace)

    outp = np.empty((B, C_OUT, T, H, W), np.float32)
    for c in range(N_CORES):
        b, q = divmod(c, 4)
        r = res.results[c]["out"]                     # [bt, dt, dh, co, bh, w]
        r = r.astype(np.float32)
        r = r.transpose(3, 0, 1, 4, 2, 5).reshape(C_OUT, TSH, H, W)
        outp[b, :, q * TSH:(q + 1) * TSH] = r
    return outp, res


def kernel(x, weight, bias):
    outp, _ = run(x, weight, bias, trace=False)
    return outp

